# revision 1
# baseline (speedup 1.0000x reference)
"""Trainium2 Bass kernel: batched single-channel 7x7 conv2d (stride 1, pad 3).

Strategy
--------
Pure data parallel over batch: 64 images -> 8 cores x 8 images.

Per core, the 2D conv is computed on the TensorEngine as 7 accumulating
matmuls per output tile: for each horizontal tap v, a banded-Toeplitz
matrix T_v ([K=128 input rows, M<=122 output rows], T_v[k,m] = W[u,v]
with u = d + k - m) performs the full 7-tap *vertical* convolution of a
128-row image strip in one matmul; the 7 horizontal taps come from
column-shifted access patterns on the same SBUF strip, accumulated in
PSUM. Zero padding is realized by clipping the Toeplitz band (rows) and
by narrowing the out/rhs column ranges (columns) - no zero-fill needed.

Row tiling: each 128-row strip yields 122 complete output rows; 9 strips
cover a 1024-row image. Three Toeplitz variants (band offsets d = 3, 0,
-77) handle the first / interior / last strips.

Inputs are cast to bf16 on host (PSUM accumulates fp32); output is fp32.
"""

import os
import numpy as np
import ml_dtypes
from contextlib import ExitStack

import concourse.bass as bass
import concourse.tile as tile
from concourse import bacc, mybir
from concourse.bass_utils import run_bass_kernel_spmd

N_CORES = 8
B, H, W_IMG = 64, 1024, 1024
B_LOC = B // N_CORES
KS, PAD = 7, 3
TILE_ROWS = 128
# A 128-row input strip yields up to 122 complete output rows, but SBUF->DRAM
# stores only spread across all 16 SDMA engines when the partition count is a
# multiple of 8 (measured: 122/124-partition stores pin to 2 engines at
# ~50 GB/s; 120/112/96/64 run at ~400 GB/s). 120 keeps the same strip count.
OUT_ROWS = 120
COL_BLOCK = 512


def row_tiles(h):
    """Per-image row tiling: list of (A, O, M, d).

    A: first input row loaded (128 rows [A, A+128) always in-bounds),
    O: first output row, M: number of output rows, d: Toeplitz band
    offset (= A - O + PAD).
    """
    tiles = []
    o = 0
    while o < h:
        m = min(OUT_ROWS, h - o)
        a = min(max(o - PAD, 0), h - TILE_ROWS)
        tiles.append((a, o, m, a - o + PAD))
        o += m
    return tiles


def col_blocks(w):
    blocks = []
    c = 0
    while c < w:
        blocks.append((c, min(c + COL_BLOCK, w)))
        c += COL_BLOCK
    return blocks


def tap_ranges(c0, c1, w):
    """For each tap v: (out_lo, out_hi, shift) with rhs cols = out cols + shift."""
    out = []
    for v in range(KS):
        sh = v - PAD
        lo = max(c0, -sh)
        hi = min(c1, w - sh)
        out.append((lo, hi, sh))
    return out


def build_toeplitz(w7, d_list, np_dtype):
    """Packed Toeplitz weights [128, len(d_list)*7*128].

    Slice [:, (di*7+v)*128:(di*7+v+1)*128][k, m] = W[d+k-m, v] (0 if out
    of band). Column m of slice (di, v) is output row m of a strip with
    band offset d = d_list[di].
    """
    n = len(d_list)
    t = np.zeros((TILE_ROWS, n * KS, TILE_ROWS), dtype=np.float32)
    k = np.arange(TILE_ROWS)[:, None]
    m = np.arange(TILE_ROWS)[None, :]
    for di, d in enumerate(d_list):
        u = d + k - m
        mask = (u >= 0) & (u < KS)
        uu = np.clip(u, 0, KS - 1)
        for v in range(KS):
            t[:, di * KS + v, :] = np.where(mask, w7[uu, v], 0.0)
    flat = t.reshape(TILE_ROWS, n * KS * TILE_ROWS)
    # trailing zero block: operands for the PE warm-up matmuls (avoids an
    # on-device memset on the critical path at kernel start)
    flat = np.concatenate([flat, np.zeros((TILE_ROWS, COL_BLOCK), np.float32)], axis=1)
    return np.ascontiguousarray(flat.astype(np_dtype))


def build_program(b_loc, h, w, in_dt=mybir.dt.bfloat16):
    """Build + compile the per-core Bass program. Returns (nc, d_list)."""
    tiles = row_tiles(h)
    blocks = col_blocks(w)
    d_list = sorted({d for (_, _, _, d) in tiles})
    d_idx = {d: i for i, d in enumerate(d_list)}
    n_toep = len(d_list) * KS

    nc = bacc.Bacc("TRN2", target_bir_lowering=False, debug=False)
    x_d = nc.dram_tensor("x", [b_loc, h, w], in_dt, kind="ExternalInput").ap()
    t_d = nc.dram_tensor(
        "toep", [TILE_ROWS, n_toep * TILE_ROWS + COL_BLOCK], in_dt, kind="ExternalInput"
    ).ap()
    y_d = nc.dram_tensor("y", [b_loc, h, w], mybir.dt.float32, kind="ExternalOutput").ap()

    with tile.TileContext(nc) as tc, ExitStack() as ctx:
        wpool = ctx.enter_context(tc.tile_pool(name="wpool", bufs=1))
        inpool = ctx.enter_context(tc.tile_pool(name="inpool", bufs=6))
        outpool = ctx.enter_context(tc.tile_pool(name="outpool", bufs=6))
        pspool = ctx.enter_context(tc.tile_pool(name="pspool", bufs=8, space="PSUM"))

        wt = wpool.tile([TILE_ROWS, n_toep * TILE_ROWS + COL_BLOCK], in_dt, name="wt")
        nc.sync.dma_start(wt[:], t_d[:])

        # fp32r matmuls require even PSUM free-dim offset/size, which the
        # ragged edge-tap ranges violate; pad 4 zero columns each side so
        # every tap is full-width instead.
        padc = 4 if in_dt == mybir.dt.float32r else 0

        # Pre-warm the PE's HAM clock gate during the initial DMA head so the
        # first real matmuls run at 2.4 GHz instead of 1.2 GHz. Operands are
        # the zero block shipped at the tail of the toep DMA.
        warm = wt[:, n_toep * TILE_ROWS :]
        wps = pspool.tile([TILE_ROWS, COL_BLOCK], mybir.dt.float32, name="wps", tag="ps")
        N_WARM = 16
        for i in range(N_WARM):
            nc.tensor.matmul(
                wps[:],
                warm[:, :TILE_ROWS],
                warm[:],
                start=(i == 0),
                stop=(i == N_WARM - 1),
            )

        for bi in range(b_loc):
            for (a, o, m, d) in tiles:
                xt = inpool.tile([TILE_ROWS, w + 2 * padc], in_dt, name="xt", tag="xt")
                if padc:
                    # memset rejects the float32r value type; zero the pad
                    # columns through a plain-f32 view of the same bytes
                    nc.gpsimd.memset(xt[:, :padc].bitcast(mybir.dt.float32), 0.0)
                    nc.gpsimd.memset(xt[:, w + padc :].bitcast(mybir.dt.float32), 0.0)
                # loads on the scalar HWDGE ring, stores on sync's: two FIFOs
                nc.scalar.dma_start(xt[:, padc : w + padc], x_d[bi, a : a + TILE_ROWS, :])
                ot = outpool.tile([TILE_ROWS, w], mybir.dt.float32, name="ot", tag="ot")
                pss = [
                    pspool.tile(
                        [TILE_ROWS, COL_BLOCK], mybir.dt.float32, name="ps", tag="ps"
                    )
                    for _ in blocks
                ]
                # tap-major: both column blocks reuse one lhsT back-to-back
                order = [PAD] + [v for v in range(KS) if v != PAD]
                for i, v in enumerate(order):
                    lhsT = wt[:, (d_idx[d] * KS + v) * TILE_ROWS :][:, :m]
                    for cb, (c0, c1) in enumerate(blocks):
                        cw = c1 - c0
                        if padc:
                            lo, hi, sh = c0, c1, v - PAD
                            out_ap = pss[cb][:m, :cw]
                        else:
                            lo, hi, sh = tap_ranges(c0, c1, w)[v]
                            out_ap = pss[cb][:m, lo - c0 : hi - c0]
                        nc.tensor.matmul(
                            out_ap,
                            lhsT,
                            xt[:, padc + lo + sh : padc + hi + sh],
                            start=(i == 0),
                            stop=(i == KS - 1),
                        )
                for cb, (c0, c1) in enumerate(blocks):
                    cw = c1 - c0
                    if cb % 2 == 0:
                        nc.vector.tensor_copy(ot[:m, c0:c1], pss[cb][:m, :cw])
                    else:
                        nc.scalar.copy(ot[:m, c0:c1], pss[cb][:m, :cw])
                nc.sync.dma_start(y_d[bi, o : o + m, :], ot[:m, :])

    nc.compile()
    return nc, d_list


_CACHE = {}


def _get_program(b_loc, h, w, in_dt):
    key = (b_loc, h, w, in_dt)
    if key not in _CACHE:
        _CACHE[key] = build_program(b_loc, h, w, in_dt=in_dt)
    return _CACHE[key]


# Measured on HW (full 64x1024x1024 problem, 8 cores):
#   bfloat16: 240 us/core, rel err 2.0e-3
#   float32r: 256 us/core, rel err 1.2e-4  (fp32 storage, TF32-like matmul,
#             1 cycle/row at N>=256 - near the bf16 PE rate)
# float32r default: 16x better accuracy for ~7% more time.
IN_DT = mybir.dt.float32r


def kernel(X, W, _trace=False, _trace_dir=None):
    X = np.asarray(X, dtype=np.float32)
    W = np.asarray(W, dtype=np.float32)
    assert X.shape == (B, H, W_IMG) and W.shape == (KS, KS)

    nc, d_list = _get_program(B_LOC, H, W_IMG, IN_DT)
    np_dt = mybir.dt.np(IN_DT)
    x_cast = X.astype(np_dt) if np_dt != np.float32 else X
    toep = build_toeplitz(W, d_list, np_dt)
    in_maps = [
        {"x": x_cast[c * B_LOC : (c + 1) * B_LOC], "toep": toep}
        for c in range(N_CORES)
    ]
    res = run_bass_kernel_spmd(
        nc, in_maps, list(range(N_CORES)), trace=_trace, tmpdir=_trace_dir
    )
    out = np.concatenate([res.results[c]["y"] for c in range(N_CORES)], axis=0)
    if _trace:
        return out, res
    return out



# revision 5
# speedup vs baseline: 1.2386x; 1.2386x over previous
"""Trainium2 Bass kernel: batched single-channel 7x7 conv2d (stride 1, pad 3).

Strategy (v3: 8-way 64x32 PE array tiling, padded input, batched DMA)
---------------------------------------------------------------------
Pure data parallel over batch: 64 images -> 8 cores x 8 images.

The 7-tap vertical band Toeplitz matmul populates only a 7-wide band of
the stationary operand, so a full 128x128 matmul wastes ~94% of the PE
array. v3 reconfigures the array into 8 independent 64x32 subarrays
(tile_position): each computes a (K=38 input rows, M=32 output rows)
band matmul. M=32 exactly fills a 32-partition PSUM column group, so
PSUM banks pack 128 consecutive output rows gap-free.

Host pads each image to [1032, 1032] bf16 (3 zero rows/cols of conv
padding baked in), which makes every window/tap uniform: K=38-row
windows at stride 32, single band variant (d=0), every matmul N=512.

Pass group = 8 windows (4 consecutive 32-row output blocks per SBUF
half) x 2 column blocks = 16 tasks on 8 subarray positions (2 tasks,
i.e. 2x7 accumulating matmuls, serialize per position; 8 positions run
concurrently). PSUM bank (b, h) = windows of half h at column block b =
[128 consecutive output rows x 512 cols].

DMA is batched hard (the HWDGE queue pays ~0.6-0.8 us issue per
instruction): 2 loads per group (overlapping-window access pattern,
[40, 4x1032] bf16 ~330 KB) and 1 store per group ([128, 2048] bf16
512 KB, 4D DRAM access pattern). ~100 DMAs per core total.

I/O bf16 both ways (kernel is HBM-bound otherwise): ~20.3 MB in +
16.8 MB out per core. Host upcasts the output to fp32.
"""

import numpy as np
import ml_dtypes
from contextlib import ExitStack

import concourse.bass as bass
import concourse.tile as tile
from concourse import bacc, mybir
from concourse.bass_utils import run_bass_kernel_spmd

N_CORES = 8
B, H, W_IMG = 64, 1024, 1024
B_LOC = B // N_CORES
KS, PAD = 7, 3
HP = WP = 1032          # padded image dims (3 top/left, 5/5 bottom/right zeros)
WIN_K = 38              # input rows feeding one 32-row output block
K_LOAD = 40             # rows loaded per window (partition count multiple of 8)
WIN_M = 32              # output rows per window
COL_BLOCK = 512
N_WIN = H // WIN_M      # 32 windows per image
WINS_PER_GROUP = 8      # 4 per SBUF half
GROUPS_PER_IMG = N_WIN // WINS_PER_GROUP  # 4


def build_toeplitz(w7, np_dt):
    """Band weights [128, 7*32 + 512] bf16, replicated in partition halves.

    Slice [64h : 64h+38, 32v : 32v+32][k, m] = W[k-m, v] (0 outside the
    band). Trailing 512-col zero block feeds the PE warm-up matmuls.
    """
    t = np.zeros((128, KS * WIN_M), dtype=np.float32)
    k = np.arange(WIN_K)[:, None]
    m = np.arange(WIN_M)[None, :]
    u = k - m
    mask = (u >= 0) & (u < KS)
    uu = np.clip(u, 0, KS - 1)
    for v in range(KS):
        band = np.where(mask, w7[uu, v], 0.0)
        t[0:WIN_K, v * WIN_M : (v + 1) * WIN_M] = band
        t[64 : 64 + WIN_K, v * WIN_M : (v + 1) * WIN_M] = band
    t = np.concatenate([t, np.zeros((128, COL_BLOCK), np.float32)], axis=1)
    return np.ascontiguousarray(t.astype(np_dt))


def build_program(b_loc, in_dt=mybir.dt.bfloat16):
    n_toep = KS * WIN_M

    nc = bacc.Bacc("TRN2", target_bir_lowering=False, debug=False)
    x_d = nc.dram_tensor("x", [b_loc, HP, WP], in_dt, kind="ExternalInput").ap()
    t_d = nc.dram_tensor(
        "toep", [128, n_toep + COL_BLOCK], in_dt, kind="ExternalInput"
    ).ap()
    y_d = nc.dram_tensor("y", [b_loc, H, W_IMG], in_dt, kind="ExternalOutput").ap()

    with tile.TileContext(nc) as tc, ExitStack() as ctx:
        wpool = ctx.enter_context(tc.tile_pool(name="wpool", bufs=1))
        inpool = ctx.enter_context(tc.tile_pool(name="inpool", bufs=4))
        outpool = ctx.enter_context(tc.tile_pool(name="outpool", bufs=4))
        pspool = ctx.enter_context(tc.tile_pool(name="pspool", bufs=8, space="PSUM"))

        wt = wpool.tile([128, n_toep + COL_BLOCK], in_dt, name="wt")
        nc.sync.dma_start(wt[:], t_d[:])

        # Pre-warm the PE HAM clock gate during the initial DMA head.
        warm = wt[:, n_toep:]
        wps = pspool.tile([128, COL_BLOCK], mybir.dt.float32, name="wps", tag="ps")
        N_WARM = 16
        for i in range(N_WARM):
            nc.tensor.matmul(
                wps[:], warm[:, :128], warm[:], start=(i == 0), stop=(i == N_WARM - 1)
            )

        order = [PAD] + [v for v in range(KS) if v != PAD]
        for g in range(b_loc * GROUPS_PER_IMG):
            bi = g // GROUPS_PER_IMG
            w0 = WINS_PER_GROUP * (g % GROUPS_PER_IMG)  # first window of group
            o0 = WIN_M * w0                             # first output row

            # One overlapping-window load per SBUF half: 4 windows of
            # K_LOAD rows at stride 32 -> [40, 4*1032].
            wtile = inpool.tile([128, 4 * WP], in_dt, name="wtile", tag="in")
            for h in range(2):
                r0 = WIN_M * (w0 + 4 * h)
                src = bass.AP(
                    x_d.tensor,
                    bi * HP * WP + r0 * WP,
                    [[WP, K_LOAD], [WIN_M * WP, 4], [1, WP]],
                )
                nc.scalar.dma_start(wtile[64 * h : 64 * h + K_LOAD, :], src)

            # PSUM bank (b, h) <- half h's 4 windows at column block b.
            pss = [
                pspool.tile([128, COL_BLOCK], mybir.dt.float32, name="ps", tag="ps")
                for _ in range(4)
            ]
            for vi, v in enumerate(order):
                # cb outer / s inner: consecutive matmuls target different
                # subarrays (matmul starts are pc-monotone, so a same-
                # subarray pair would stall the whole queue).
                for cb in range(2):
                    for s in range(WINS_PER_GROUP):  # subarray position
                        h, wq = s // 4, s % 4
                        nc.tensor.matmul(
                            pss[2 * cb + h][32 * wq : 32 * wq + WIN_M, :],
                            wt[64 * h : 64 * h + WIN_K, v * WIN_M : (v + 1) * WIN_M],
                            wtile[
                                64 * h : 64 * h + WIN_K,
                                WP * wq + COL_BLOCK * cb + v :
                                WP * wq + COL_BLOCK * cb + v + COL_BLOCK,
                            ],
                            start=(vi == 0),
                            stop=(vi == len(order) - 1),
                            tile_position=(64 * h, 32 * wq),
                        )

            # Evacuate 4 banks into one [128, 2048] bf16 tile; free layout
            # (cb, h, c) so a single 4D-DRAM-AP store covers the group.
            ot = outpool.tile([128, 4 * COL_BLOCK], in_dt, name="ot", tag="ot")
            for cb in range(2):
                for h in range(2):
                    dst = ot[:, (2 * cb + h) * COL_BLOCK : (2 * cb + h + 1) * COL_BLOCK]
                    if h == 0:
                        nc.vector.tensor_copy(dst, pss[2 * cb + h][:])
                    else:
                        nc.scalar.copy(dst, pss[2 * cb + h][:])
            dst = bass.AP(
                y_d.tensor,
                bi * H * W_IMG + o0 * W_IMG,
                # iteration order (p, cb, h, c)
                [[W_IMG, 128], [COL_BLOCK, 2], [128 * W_IMG, 2], [1, COL_BLOCK]],
            )
            nc.sync.dma_start(dst, ot[:])

    nc.compile()
    return nc


_CACHE = {}


def _get_program(b_loc, in_dt):
    key = (b_loc, in_dt)
    if key not in _CACHE:
        _CACHE[key] = build_program(b_loc, in_dt=in_dt)
    return _CACHE[key]


IN_DT = mybir.dt.bfloat16


def kernel(X, W, _trace=False, _trace_dir=None):
    X = np.asarray(X, dtype=np.float32)
    W = np.asarray(W, dtype=np.float32)
    assert X.shape == (B, H, W_IMG) and W.shape == (KS, KS)

    nc = _get_program(B_LOC, IN_DT)
    np_dt = mybir.dt.np(IN_DT)
    xp = np.zeros((B, HP, WP), dtype=np_dt)
    xp[:, PAD : PAD + H, PAD : PAD + W_IMG] = X.astype(np_dt)
    toep = build_toeplitz(W, np_dt)
    in_maps = [
        {"x": xp[c * B_LOC : (c + 1) * B_LOC], "toep": toep}
        for c in range(N_CORES)
    ]
    res = run_bass_kernel_spmd(
        nc, in_maps, list(range(N_CORES)), trace=_trace, tmpdir=_trace_dir
    )
    out = np.concatenate(
        [res.results[c]["y"].astype(np.float32) for c in range(N_CORES)], axis=0
    )
    if _trace:
        return out, res
    return out
